# revision 1
# baseline (speedup 1.0000x reference)
"""Bahdanau additive attention on 8 Trainium2 NeuronCores.

Problem: B=32, S=1024, H=1024 fp32.
  U_h   = dec @ U_w.T                    [B, H]
  W_s   = enc @ W_w.T                    [B, S, H]
  att   = tanh(U_h[:,None,:] + W_s) @ v  [B, S]
  alpha = softmax(att, axis=1)
  ctx   = einsum('bs,bsh->bh', alpha, enc)

Sharding: data-parallel over B across 8 cores (4 batches per core),
U_w / W_w / v_w replicated.

Per-core kernel (all matmuls in bf16 with fp32 PSUM accumulation):
  - enc streamed in [128,1024] chunks on the sync HWDGE queue, cast to bf16
    (kept as encN for the context matmul), written back to an internal DRAM
    bf16 copy on the gpsimd SWDGE queue, and transposed into encT [h,s] by
    8 large DRAM->SBUF xbar DMA-transposes per batch. The PE array does
    only matmuls in the steady state.
  - W_sT tiles [o=128, s=512] = W_wT.T @ encT accumulated over 8 h-tiles.
  - W_wT / U_wT built once at startup with PE transposes (fills PE idle
    time while the first batch streams in).
  - ScalarE tanh with per-partition bias U_hT[o] fused on PSUM evacuation.
  - score reduction over o as a PE matvec with v (accumulated over o-tiles).
  - per-batch softmax on a partition-0 staging row; alpha bounced through
    the alpha output buffer in DRAM to transpose it for the context matmul.
  - context matmuls for batch b run interleaved inside batch b+1's main
    loop so only the last batch's epilogue is serial.
"""

import numpy as np
from contextlib import ExitStack

import concourse.bacc as bacc
import concourse.mybir as mybir
import concourse.tile as tile
from concourse import masks
from concourse.bass_utils import run_bass_kernel_spmd

N_CORES = 8
B = 32
B_L = B // N_CORES  # 4 batches per core
S = 1024
H = 1024
P = 128
NT = 8  # 1024 / 128 tiles
F32 = mybir.dt.float32
BF16 = mybir.dt.bfloat16
AF = mybir.ActivationFunctionType


def _emit(tc, stop_after="full"):
    nc = tc.nc
    dec = nc.dram_tensor("dec", [B_L, H], F32, kind="ExternalInput").ap()
    enc = nc.dram_tensor("enc", [B_L, S, H], F32, kind="ExternalInput").ap()
    U_w = nc.dram_tensor("U_w", [H, H], F32, kind="ExternalInput").ap()
    W_w = nc.dram_tensor("W_w", [H, H], F32, kind="ExternalInput").ap()
    v_w = nc.dram_tensor("v_w", [H], F32, kind="ExternalInput").ap()
    ctx_out = nc.dram_tensor("ctx", [B_L, H], F32, kind="ExternalOutput").ap()
    alpha_out = nc.dram_tensor("alpha", [B_L, S], F32, kind="ExternalOutput").ap()

    ctx = ExitStack()
    const = ctx.enter_context(tc.tile_pool(name="const", bufs=1))
    natp = ctx.enter_context(tc.tile_pool(name="nat", bufs=4))
    encTp = ctx.enter_context(tc.tile_pool(name="encT", bufs=2))
    tanhp = ctx.enter_context(tc.tile_pool(name="tanh", bufs=3))
    stgp = ctx.enter_context(tc.tile_pool(name="stg", bufs=1))
    mmps = ctx.enter_context(tc.tile_pool(name="mmps", bufs=4, space="PSUM"))
    trps = ctx.enter_context(tc.tile_pool(name="trps", bufs=2, space="PSUM"))
    attps = ctx.enter_context(tc.tile_pool(name="attps", bufs=1, space="PSUM"))

    identb = const.tile([P, P], BF16)
    masks.make_identity(nc, identb[:])
    identf = const.tile([P, P], F32)  # only needed by the b3 alpha epilogue;
    # built later so it never gates the prologue

    # --- decT via one fast natural load + tiny bf16 PE transposes ---
    dec_nat = const.tile([B_L, H], F32)
    nc.sync.dma_start(dec_nat[:], dec[:])
    dec_b16 = const.tile([B_L, H], BF16)
    nc.vector.tensor_copy(dec_b16[:], dec_nat[:])
    decT = const.tile([P, NT, B_L], BF16)
    for k in range(NT):
        ps = mmps.tile([P, B_L], BF16, tag="mm", name="dec_ps")
        nc.tensor.transpose(
            ps[:], dec_b16[:, k * P : (k + 1) * P], identb[0:B_L, 0:B_L]
        )
        nc.vector.tensor_copy(decT[:, k, :], ps[:])

    # --- weights: load natural (SWDGE), cast bf16, PE-transpose ---
    W_wT = const.tile([P, NT, H], BF16)  # [h_in, h_tile, o]
    U_wT = encTp.tile([P, NT, H], BF16, tag="encT")
    encN = const.tile([P, B_L, NT, H], BF16)  # [s_in, b, s_tile, h]
    alphaT = const.tile([P, NT, B_L], BF16)
    encT_cur = encTp.tile([P, NT, S], BF16, tag="encT", name="encT_0")

    def transpose_in(dst, src_dram):
        # src [1024, 1024] row-major -> dst[p, j, o] = src[o, 128*j + p]
        for i in range(NT):  # row tile of src (o)
            natt = natp.tile([P, H], F32, tag="nat")
            nc.gpsimd.dma_start(natt[:], src_dram[i * P : (i + 1) * P, :])
            natb = natp.tile([P, H], BF16, tag="natb")
            nc.vector.tensor_copy(natb[:], natt[:])
            for g in range(2):  # groups of 4 h-tiles through one PSUM bank
                ps = attps.tile([P, 4, P], BF16, tag=f"att{g}", name="wtr_ps")
                for jj in range(4):
                    j = 4 * g + jj
                    nc.tensor.transpose(
                        ps[:, jj, :], natb[:, j * P : (j + 1) * P], identb[:]
                    )
                nc.vector.tensor_copy(
                    dst[:, 4 * g : 4 * g + 4, i * P : (i + 1) * P], ps[:]
                )

    def emit_batch_load(b):
        """Stream enc[b]: load f32 chunks, cast bf16 into encN."""
        for k in range(NT):
            natt = natp.tile([P, H], F32, tag="nat")
            nc.sync.dma_start(natt[:], enc[b, k * P : (k + 1) * P, :])
            nc.vector.tensor_copy(encN[:, b, k, :], natt[:])

    def emit_transpose_group(encT_b, b, k, g):
        """PE-transpose 4 h-tiles of chunk (b, k) into encT_b."""
        ps = trps.tile([P, 4, P], BF16, tag="tr")
        for jj in range(4):
            j = 4 * g + jj
            nc.tensor.transpose(
                ps[:, jj, :], encN[:, b, k, j * P : (j + 1) * P], identb[:]
            )
        nc.vector.tensor_copy(
            encT_b[:, 4 * g : 4 * g + 4, k * P : (k + 1) * P], ps[:]
        )

    def emit_ctx(b):
        """Context matmuls + evacuation + output DMA for batch b."""
        ps = attps.tile([P, 512], F32, tag="att1", name="ctx_ps")
        for k in range(NT):
            lhsT = alphaT[:, k, b : b + 1]
            for c in range(2):
                nc.tensor.matmul(
                    ps[32 * c : 32 * c + 1, :],
                    lhsT,
                    encN[:, b, k, c * 512 : (c + 1) * 512],
                    start=(k == 0),
                    stop=(k == NT - 1),
                    tile_position=(0, 32 * c),
                )
        ctx_stg = stgp.tile([1, H], F32, tag="ctxstg")
        for c in range(2):
            nc.vector.tensor_copy(
                ctx_stg[0:1, c * 512 : (c + 1) * 512],
                ps[32 * c : 32 * c + 1, :],
            )
        nc.gpsimd.dma_start(ctx_out[b : b + 1, :], ctx_stg[0:1, :])

    if stop_after == "setup":
        dbg = stgp.tile([1, S], F32, tag="ctxstg")
        nc.gpsimd.memset(dbg[:], 0.0)
        for b in range(B_L):
            nc.sync.dma_start(alpha_out[b : b + 1, :], dbg[0:1, :])
            nc.sync.dma_start(ctx_out[b : b + 1, :], dbg[0:1, 0:H])
        ctx.close()
        return

    # --- interleaved prologue: batch-0 chunks and W chunks alternate so
    # VectorE's casts/evacuations fill each other's DMA-wait gaps ---
    for k in range(NT):
        natt0 = natp.tile([P, H], F32, tag="nat", name="natt0")
        nc.sync.dma_start(natt0[:], enc[0, k * P : (k + 1) * P, :])
        nc.vector.tensor_copy(encN[:, 0, k, :], natt0[:])
        nattW = natp.tile([P, H], F32, tag="nat", name="nattW")
        nc.gpsimd.dma_start(nattW[:], W_w[k * P : (k + 1) * P, :])
        natbW = natp.tile([P, H], BF16, tag="natb", name="natbW")
        nc.vector.tensor_copy(natbW[:], nattW[:])
        for g in range(2):
            emit_transpose_group(encT_cur, 0, k, g)
        for g in range(2):
            psW = attps.tile([P, 4, P], BF16, tag=f"att{g}", name="wtr_ps")
            for jj in range(4):
                j = 4 * g + jj
                nc.tensor.transpose(
                    psW[:, jj, :], natbW[:, j * P : (j + 1) * P], identb[:]
                )
            nc.vector.tensor_copy(
                W_wT[:, 4 * g : 4 * g + 4, k * P : (k + 1) * P], psW[:]
            )

    transpose_in(U_wT, U_w)

    # --- U_hT[o, b] = sum_h U_wT[h, o].T @ decT[h, b], per o-tile ---
    U_hT = const.tile([P, NT, B_L], F32)
    for i in range(NT):
        ps = mmps.tile([P, B_L], F32, tag="mm")
        for j in range(NT):
            nc.tensor.matmul(
                ps[:],
                U_wT[:, j, i * P : (i + 1) * P],
                decT[:, j, :],
                start=(j == 0),
                stop=(j == NT - 1),
            )
        nc.vector.tensor_copy(U_hT[:, i, :], ps[:])

    # --- persistent bf16 natural-layout copy of enc (for the context matmul)
    # v constant + f32 identity: emitted after the prologue so their slow
    # DMAs / gpsimd work never stall the batch-0 evacuation chain
    vT_f = const.tile([P, NT], F32)
    nc.sync.dma_start(vT_f[:], v_w.rearrange("(t p) -> p t", p=P))
    vT = const.tile([P, NT], BF16)
    nc.vector.tensor_copy(vT[:], vT_f[:])
    masks.make_identity(nc, identf[:])

    for b in range(B_L):
        encT_next = None
        if b + 1 < B_L:
            encT_next = encTp.tile(
                [P, NT, S], BF16, tag="encT", name=f"encT_{b + 1}"
            )
            emit_batch_load(b + 1)

        att_ps = attps.tile([P, 512], F32, tag="att0", name="att_ps")
        tanh_prev = None
        for i in range(NT):
            ps = [
                mmps.tile([P, 512], F32, tag="mm", name=f"mm_ps{c2}")
                for c2 in range(2)
            ]
            for j in range(NT):
                lhsT = W_wT[:, j, i * P : (i + 1) * P]
                for c in range(2):
                    nc.tensor.matmul(
                        ps[c][:],
                        lhsT,
                        encT_cur[:, j, c * 512 : (c + 1) * 512],
                        start=(j == 0),
                        stop=(j == NT - 1),
                    )
            # previous batch's context matmuls, once its softmax surely landed
            if i == 3 and 0 < b < B_L - 1:
                emit_ctx(b - 1)
            # next batch's enc transposes, interleaved into the PE stream
            if encT_next is not None:
                for g in range(2):
                    emit_transpose_group(encT_next, b + 1, i, g)
            # v-matvec for previous o-tile (tanh ready; keeps PE rolling).
            # The two s-chunks go to column groups 0 and 32 -> concurrent.
            if tanh_prev is not None:
                ip, th = tanh_prev
                for c in range(2):
                    nc.tensor.matmul(
                        att_ps[32 * c : 32 * c + 1, :],
                        vT[:, ip : ip + 1],
                        th[:, c * 512 : (c + 1) * 512],
                        start=(ip == 0),
                        stop=(ip == NT - 1),
                        tile_position=(0, 32 * c),
                    )
            th = tanhp.tile([P, 1024], BF16, tag="tanh")
            for c in range(2):
                nc.scalar.activation(
                    th[:, c * 512 : (c + 1) * 512],
                    ps[c][:],
                    AF.Tanh,
                    bias=U_hT[:, i, b : b + 1],
                    scale=1.0,
                )
            tanh_prev = (i, th)

        ip, th = tanh_prev
        for c in range(2):
            nc.tensor.matmul(
                att_ps[32 * c : 32 * c + 1, :],
                vT[:, ip : ip + 1],
                th[:, c * 512 : (c + 1) * 512],
                start=(ip == 0),
                stop=(ip == NT - 1),
                tile_position=(0, 32 * c),
            )
        if b == B_L - 1:
            emit_ctx(b - 1)

        # --- per-batch epilogue: att evac, softmax, alpha out + transpose ---
        att_stg = stgp.tile([1, S], F32, tag="attstg")
        for c in range(2):
            nc.vector.tensor_copy(
                att_stg[0:1, c * 512 : (c + 1) * 512],
                att_ps[32 * c : 32 * c + 1, :],
            )
        if stop_after != "phase1":
            smax = stgp.tile([1, 1], F32, tag="smax")
            nc.vector.reduce_max(smax[:], att_stg[:], axis=mybir.AxisListType.X)
            negmax = stgp.tile([1, 1], F32, tag="negmax")
            nc.vector.tensor_scalar_mul(negmax[:], smax[:], -1.0)
            exp_stg = stgp.tile([1, S], F32, tag="expstg")
            ssum = stgp.tile([1, 1], F32, tag="ssum")
            nc.scalar.activation(
                exp_stg[:],
                att_stg[:],
                AF.Exp,
                bias=negmax[:],
                scale=1.0,
                accum_out=ssum[:],
            )
            srec = stgp.tile([1, 1], F32, tag="srec")
            nc.vector.reciprocal(srec[:], ssum[:])
            alpha_stg = stgp.tile([1, S], F32, tag="alphastg")
            nc.vector.tensor_scalar_mul(alpha_stg[:], exp_stg[:], srec[:])
        else:
            alpha_stg = att_stg
        nc.gpsimd.dma_start(alpha_out[b : b + 1, :], alpha_stg[0:1, :])
        if b < B_L - 1:
            # bounce through DRAM to transpose alpha (latency hidden here)
            alphaT_f = stgp.tile([P, NT], F32, tag="alphaTf")
            nc.gpsimd.dma_start(
                alphaT_f[:], alpha_out[b].rearrange("(k p) -> p k", p=P)
            )
            nc.vector.tensor_copy(alphaT[:, :, b], alphaT_f[:])
        else:
            # last batch: PE is idle; transpose alpha on the array instead
            ps = mmps.tile([P, NT], F32, tag="mm", name="alpha_ps")
            for k in range(NT):
                nc.tensor.transpose(
                    ps[:, k : k + 1],
                    alpha_stg[0:1, k * P : (k + 1) * P],
                    identf[0:1, 0:1],
                )
            nc.vector.tensor_copy(alphaT[:, :, b], ps[:])
        if encT_next is not None:
            encT_cur = encT_next

    emit_ctx(B_L - 1)
    ctx.close()


_CACHED = None


def _build(stop_after="full"):
    global _CACHED
    if _CACHED is None:
        nc = bacc.Bacc("TRN2", target_bir_lowering=False, debug=False)
        with tile.TileContext(nc) as tc:
            _emit(tc, stop_after=stop_after)
        nc.compile()
        _CACHED = nc
    return _CACHED


def kernel(
    decoder_hidden: np.ndarray,
    encoder_outputs: np.ndarray,
    U_w: np.ndarray,
    W_w: np.ndarray,
    v_w: np.ndarray,
):
    dec = np.ascontiguousarray(np.asarray(decoder_hidden, dtype=np.float32))
    enc = np.ascontiguousarray(np.asarray(encoder_outputs, dtype=np.float32))
    U = np.ascontiguousarray(np.asarray(U_w, dtype=np.float32))
    W = np.ascontiguousarray(np.asarray(W_w, dtype=np.float32))
    v = np.ascontiguousarray(np.asarray(v_w, dtype=np.float32))

    nc = _build()
    in_maps = []
    for c in range(N_CORES):
        sl = slice(c * B_L, (c + 1) * B_L)
        in_maps.append(
            {"dec": dec[sl], "enc": enc[sl], "U_w": U, "W_w": W, "v_w": v}
        )
    res = run_bass_kernel_spmd(nc, in_maps, core_ids=list(range(N_CORES)))
    context = np.concatenate([res.results[c]["ctx"] for c in range(N_CORES)], axis=0)
    alpha = np.concatenate([res.results[c]["alpha"] for c in range(N_CORES)], axis=0)
    return (context.astype(np.float32), alpha.astype(np.float32))



# revision 2
# speedup vs baseline: 1.2594x; 1.2594x over previous
"""Bahdanau additive attention on 8 Trainium2 NeuronCores.

Problem: B=32, S=1024, H=1024 fp32.
  U_h   = dec @ U_w.T                    [B, H]
  W_s   = enc @ W_w.T                    [B, S, H]
  att   = tanh(U_h[:,None,:] + W_s) @ v  [B, S]
  alpha = softmax(att, axis=1)
  ctx   = einsum('bs,bsh->bh', alpha, enc)

Sharding: data-parallel over B across 8 cores (4 batches per core),
U_w / W_w / v_w replicated.

All dtype casts and layout transposes are done on the HOST:
  - encT  [B_L,128,NT,S] bf16: encT[b,p,j,s] = enc[b,s,128j+p]  (GEMM rhs)
  - encN  [B_L,128,NT,H] bf16: encN[b,p,k,h] = enc[b,128k+p,h]  (ctx rhs)
  - WT    [NO,128,NT,128] bf16: WT[i,p,j,oo] = W_w[128i+oo,128j+p]
  - UT    [NT,128,H] bf16:      UT[j,p,o]    = U_w[o,128j+p]
  - decT  [128,NT,B_L] bf16:    decT[p,j,b]  = dec[b,128j+p]
  - vT    [128,NT] bf16:        vT[p,t]      = v[128t+p]
so the device kernel does ONLY matmuls on the PE (no transposes, no
casts): the W_s GEMM streams encT against stationary WT tiles, ScalarE
applies tanh with per-partition bias U_hT on PSUM evacuation, the
v-matvec and per-batch softmax follow the baseline structure, and the
context matmul for batch b runs interleaved inside batch b+1's loop.
"""

import numpy as np
import ml_dtypes
from contextlib import ExitStack

import concourse.bacc as bacc
import concourse.mybir as mybir
import concourse.tile as tile
from concourse.bass_utils import run_bass_kernel_spmd

N_CORES = 8
B = 32
B_L = B // N_CORES  # 4 batches per core
S = 1024
H = 1024
P = 128
NT = 8  # 1024 / 128 tiles
F32 = mybir.dt.float32
BF16 = mybir.dt.bfloat16
AF = mybir.ActivationFunctionType
BF = ml_dtypes.bfloat16


def _emit(tc):
    nc = tc.nc
    encT_d = nc.dram_tensor("encT", [B_L, P, NT, S], BF16, kind="ExternalInput").ap()
    encN_d = nc.dram_tensor("encN", [B_L, P, NT, H], BF16, kind="ExternalInput").ap()
    WT_d = nc.dram_tensor("WT", [NT, P, NT, P], BF16, kind="ExternalInput").ap()
    UT_d = nc.dram_tensor("UT", [NT, P, H], BF16, kind="ExternalInput").ap()
    decT_d = nc.dram_tensor("decT", [P, NT, B_L], BF16, kind="ExternalInput").ap()
    vT_d = nc.dram_tensor("vT", [P, NT], BF16, kind="ExternalInput").ap()
    ident_d = nc.dram_tensor("ident", [4, 4], F32, kind="ExternalInput").ap()
    ctx_out = nc.dram_tensor("ctx", [B_L, H], F32, kind="ExternalOutput").ap()
    alpha_out = nc.dram_tensor("alpha", [B_L, S], F32, kind="ExternalOutput").ap()

    ctx = ExitStack()
    const = ctx.enter_context(tc.tile_pool(name="const", bufs=1))
    encTp = ctx.enter_context(tc.tile_pool(name="encT", bufs=2))
    encNp = ctx.enter_context(tc.tile_pool(name="encN", bufs=2))
    thp = ctx.enter_context(tc.tile_pool(name="tanh", bufs=3))
    stgp = ctx.enter_context(tc.tile_pool(name="stg", bufs=1))
    psp = ctx.enter_context(tc.tile_pool(name="ps", bufs=1, space="PSUM"))

    # --- SBUF constants ---
    W_sb = const.tile([P, NT, NT, P], BF16)  # [p, i, j, oo] 16KiB/part
    U_sb = const.tile([P, NT, H], BF16)  # [p, j, o]        16KiB/part
    dec_sb = const.tile([P, NT, B_L], BF16)
    v_sb = const.tile([P, NT], BF16)
    ident4 = const.tile([4, 4], F32)
    U_hT = const.tile([P, NT, B_L], F32)
    alphaT = const.tile([P, NT, B_L], BF16)

    # --- prologue DMAs ---
    # scalar HWDGE queue: W o-tile 0 (start-critical), dec, v, ident, U
    nc.scalar.dma_start(W_sb[:, 0, :, :], WT_d[0])
    nc.scalar.dma_start(dec_sb[:], decT_d[:])
    nc.scalar.dma_start(v_sb[:], vT_d[:])
    nc.scalar.dma_start(ident4[:], ident_d[:])
    for j in range(NT):
        nc.scalar.dma_start(U_sb[:, j, :], UT_d[j])
    # sync HWDGE queue: batch-0 encT chunks
    encT_cur = encTp.tile([P, NT, S], BF16, tag="encT", name="encT_0")
    for j in range(NT):
        nc.sync.dma_start(encT_cur[:, j, :], encT_d[0, :, j, :])
    # gpsimd SWDGE queue: remaining W o-tiles
    for i in range(1, NT):
        nc.gpsimd.dma_start(W_sb[:, i, :, :], WT_d[i])
    # scalar queue (behind U): batch-0 encN (needed at ctx(0) in batch 1)
    encN_cur = encNp.tile([P, NT, H], BF16, tag="encN", name="encN_0")
    nc.scalar.dma_start(encN_cur[:], encN_d[0])

    def emit_uh():
        """U_hT[o, b] = (dec @ U_w.T).T via out-[b, o] GEMM + tiny transposes."""
        psU = [psp.tile([P, 512], F32, tag="u", bufs=2, name=f"psU{c}") for c in range(2)]
        for j in range(NT):
            lhsT = dec_sb[:, j, :]
            for c in range(2):
                nc.tensor.matmul(
                    psU[c][0:B_L, :],
                    lhsT,
                    U_sb[:, j, 512 * c : 512 * (c + 1)],
                    start=(j == 0),
                    stop=(j == NT - 1),
                )
        U_hN = stgp.tile([B_L, H], F32, tag="uhn")
        for c in range(2):
            nc.vector.tensor_copy(U_hN[:, 512 * c : 512 * (c + 1)], psU[c][0:B_L, :])
        psT = psp.tile([P, NT, B_L], F32, tag="ctx", name="psT")
        for i2 in range(NT):
            nc.tensor.transpose(
                psT[:, i2, :], U_hN[:, i2 * P : (i2 + 1) * P], ident4[0:B_L, 0:B_L]
            )
        nc.vector.tensor_copy(U_hT[:], psT[:])

    def emit_ctx(b, encN_b):
        """Context matmuls + evacuation + output DMA for batch b."""
        ps = psp.tile([P, 512], F32, tag="ctx", name="ctx_ps")
        for k in range(NT):
            lhsT = alphaT[:, k, b : b + 1]
            for c in range(2):
                nc.tensor.matmul(
                    ps[32 * c : 32 * c + 1, :],
                    lhsT,
                    encN_b[:, k, c * 512 : (c + 1) * 512],
                    start=(k == 0),
                    stop=(k == NT - 1),
                    tile_position=(0, 32 * c),
                )
        ctx_stg = stgp.tile([1, H], F32, tag="ctxstg")
        for c in range(2):
            nc.vector.tensor_copy(
                ctx_stg[0:1, c * 512 : (c + 1) * 512], ps[32 * c : 32 * c + 1, :]
            )
        nc.gpsimd.dma_start(ctx_out[b : b + 1, :], ctx_stg[0:1, :])

    def emit_matvec(ip, th, att_ps):
        for c in range(2):
            nc.tensor.matmul(
                att_ps[32 * c : 32 * c + 1, :],
                v_sb[:, ip : ip + 1],
                th[:, c * 512 : (c + 1) * 512],
                start=(ip == 0),
                stop=(ip == NT - 1),
                tile_position=(0, 32 * c),
            )

    encN_prev = None  # encN tile of batch b-1 (consumed by ctx(b-1))
    for b in range(B_L):
        encT_next = None
        encN_next = None
        if b + 1 < B_L:
            encT_next = encTp.tile([P, NT, S], BF16, tag="encT", name=f"encT_{b+1}")
            for j in range(NT):
                nc.sync.dma_start(encT_next[:, j, :], encT_d[b + 1, :, j, :])
            encN_next = encNp.tile([P, NT, H], BF16, tag="encN", name=f"encN_{b+1}")
            nc.scalar.dma_start(encN_next[:], encN_d[b + 1])

        att_ps = psp.tile([P, 512], F32, tag="att", name="att_ps")
        tanh_prev = None
        for i in range(NT):
            ps = [
                psp.tile([P, 512], F32, tag="mm", bufs=4, name=f"mm_ps{c2}")
                for c2 in range(2)
            ]
            for j in range(NT):
                lhsT = W_sb[:, i, j, :]
                for c in range(2):
                    nc.tensor.matmul(
                        ps[c][:],
                        lhsT,
                        encT_cur[:, j, c * 512 : (c + 1) * 512],
                        start=(j == 0),
                        stop=(j == NT - 1),
                    )
            if b == 0 and i == 0:
                emit_uh()
            if b > 0 and i == 3:
                emit_ctx(b - 1, encN_prev)
            if tanh_prev is not None:
                emit_matvec(tanh_prev[0], tanh_prev[1], att_ps)
            th = thp.tile([P, S], BF16, tag="tanh")
            for c in range(2):
                nc.scalar.activation(
                    th[:, c * 512 : (c + 1) * 512],
                    ps[c][:],
                    AF.Tanh,
                    bias=U_hT[:, i, b : b + 1],
                    scale=1.0,
                )
            tanh_prev = (i, th)

        emit_matvec(tanh_prev[0], tanh_prev[1], att_ps)

        # --- per-batch epilogue: att evac, softmax, alpha out + transpose ---
        att_stg = stgp.tile([1, S], F32, tag="attstg")
        for c in range(2):
            nc.vector.tensor_copy(
                att_stg[0:1, c * 512 : (c + 1) * 512], att_ps[32 * c : 32 * c + 1, :]
            )
        smax = stgp.tile([1, 1], F32, tag="smax")
        nc.vector.reduce_max(smax[:], att_stg[:], axis=mybir.AxisListType.X)
        negmax = stgp.tile([1, 1], F32, tag="negmax")
        nc.vector.tensor_scalar_mul(negmax[:], smax[:], -1.0)
        exp_stg = stgp.tile([1, S], F32, tag="expstg")
        ssum = stgp.tile([1, 1], F32, tag="ssum")
        nc.scalar.activation(
            exp_stg[:], att_stg[:], AF.Exp, bias=negmax[:], scale=1.0, accum_out=ssum[:]
        )
        srec = stgp.tile([1, 1], F32, tag="srec")
        nc.vector.reciprocal(srec[:], ssum[:])
        alpha_stg = stgp.tile([1, S], F32, tag="alphastg")
        nc.vector.tensor_scalar_mul(alpha_stg[:], exp_stg[:], srec[:])
        nc.gpsimd.dma_start(alpha_out[b : b + 1, :], alpha_stg[0:1, :])
        if b < B_L - 1:
            # bounce through DRAM to transpose alpha (latency hidden here)
            alphaT_f = stgp.tile([P, NT], F32, tag="alphaTf")
            nc.gpsimd.dma_start(
                alphaT_f[:], alpha_out[b].rearrange("(k p) -> p k", p=P)
            )
            nc.vector.tensor_copy(alphaT[:, :, b], alphaT_f[:])
        else:
            # last batch: PE is idle; transpose alpha on the array instead
            psk = psp.tile([P, NT], F32, tag="mm", bufs=4, name="alpha_ps")
            for k in range(NT):
                nc.tensor.transpose(
                    psk[:, k : k + 1],
                    alpha_stg[0:1, k * P : (k + 1) * P],
                    ident4[0:1, 0:1],
                )
            nc.vector.tensor_copy(alphaT[:, :, b], psk[:])
        if encT_next is not None:
            encT_cur = encT_next
        encN_prev = encN_cur
        if encN_next is not None:
            encN_cur = encN_next

    emit_ctx(B_L - 1, encN_prev)
    ctx.close()


_CACHED = None


def _build():
    global _CACHED
    if _CACHED is None:
        nc = bacc.Bacc("TRN2", target_bir_lowering=False, debug=False)
        with tile.TileContext(nc) as tc:
            _emit(tc)
        nc.compile()
        _CACHED = nc
    return _CACHED


def make_in_maps(decoder_hidden, encoder_outputs, U_w, W_w, v_w):
    """Host-side layout prep: cast to bf16 and pre-transpose per core."""
    dec = np.asarray(decoder_hidden, dtype=np.float32)
    enc = np.asarray(encoder_outputs, dtype=np.float32)
    U = np.asarray(U_w, dtype=np.float32)
    W = np.asarray(W_w, dtype=np.float32)
    v = np.asarray(v_w, dtype=np.float32)

    # WT[i, p, j, oo] = W[128i+oo, 128j+p]
    WT = np.ascontiguousarray(
        W.reshape(NT, P, NT, P).transpose(0, 3, 2, 1).astype(BF)
    )
    # UT[j, p, o] = U[o, 128j+p]
    UT = np.ascontiguousarray(U.T.reshape(NT, P, H).astype(BF))
    ident = np.eye(4, dtype=np.float32)

    in_maps = []
    for c in range(N_CORES):
        sl = slice(c * B_L, (c + 1) * B_L)
        enc_sl = enc[sl]  # [B_L, S, H]
        # encT[b, p, j, s] = enc[b, s, 128j+p]
        encT = np.ascontiguousarray(
            enc_sl.transpose(0, 2, 1).reshape(B_L, NT, P, S).transpose(0, 2, 1, 3).astype(BF)
        )
        # encN[b, p, k, h] = enc[b, 128k+p, h]
        encN = np.ascontiguousarray(
            enc_sl.reshape(B_L, NT, P, H).transpose(0, 2, 1, 3).astype(BF)
        )
        # decT[p, j, b] = dec[b, 128j+p]
        decT = np.ascontiguousarray(
            dec[sl].reshape(B_L, NT, P).transpose(2, 1, 0).astype(BF)
        )
        vT = np.ascontiguousarray(v.reshape(NT, P).T.astype(BF))
        in_maps.append(
            {
                "encT": encT,
                "encN": encN,
                "WT": WT,
                "UT": UT,
                "decT": decT,
                "vT": vT,
                "ident": ident,
            }
        )
    return in_maps


def kernel(
    decoder_hidden: np.ndarray,
    encoder_outputs: np.ndarray,
    U_w: np.ndarray,
    W_w: np.ndarray,
    v_w: np.ndarray,
):
    nc = _build()
    in_maps = make_in_maps(decoder_hidden, encoder_outputs, U_w, W_w, v_w)
    res = run_bass_kernel_spmd(nc, in_maps, core_ids=list(range(N_CORES)))
    context = np.concatenate([res.results[c]["ctx"] for c in range(N_CORES)], axis=0)
    alpha = np.concatenate([res.results[c]["alpha"] for c in range(N_CORES)], axis=0)
    return (context.astype(np.float32), alpha.astype(np.float32))
